# revision 1
# baseline (speedup 1.0000x reference)
"""Trainium2 Bass kernel for dual-branch local+dilated windowed attention.

Problem: B=1, L=4096, D=512, H=8 heads (dh=64), window=±256, dilation=4.
reference returns (out_local, out_dilated), each [1, L, D] fp32.

Sharding: sequence (L) sharded across 8 cores; each core owns 512 query rows
and loads a 1024-row key slice (256-row halo each side, zero-padded at the
sequence edges).  All weights are replicated, pre-transposed, and cast to
bf16 host-side with the rmsnorm gains (and the 1/sqrt(dh) score scale)
folded in.  Per-core key-validity masks fold sequence-edge padding and the
key_padding_mask into the softmax denominator via a masked ones-column
appended to V.

On-chip pipeline per core (single NEFF, SPMD over 8 cores):
  1. rmsnorm(x) -> xhat (bf16), PE-transpose to xhat^T [D_part, t]
  2. Q/K/V projections per branch (PE, bf16, fp32 PSUM accum)
  3. local branch: per 128-key chunk, scores^T = K^T_chunk x Q -> exp (ACT)
     -> edge-triangle {0,1} mask-mults (DVE) -> attn @ [V | colmask] (PE)
     giving per-query numerators + denominator
  4. dilated branch: queries/keys regrouped by residue mod 4 (strided APs),
     where the band collapses to 2 chunks with the same triangle masks
  5. normalize by reciprocal denominator, PE-transpose, Wo projection,
     DMA out (dilated rows are written back through a strided view)
"""

import numpy as np
import ml_dtypes

L, D, H, DH = 4096, 512, 8, 64
WIN, DIL = 256, 4
EPS = 1e-6
NCORES = 8
QL = L // NCORES          # 512 queries per core
KL = QL + 2 * WIN         # 1024 keys per core (halo)
P = 128
NKC = KL // P             # 8 key chunks
NQT = QL // P             # 4 query tiles
BF16 = ml_dtypes.bfloat16

_STATE = {}


def _build_nc():
    import concourse.bacc as bacc
    import concourse.tile as tile
    import concourse.mybir as mybir
    from concourse.masks import make_identity

    f32 = mybir.dt.float32
    bf16 = mybir.dt.bfloat16
    Exp = mybir.ActivationFunctionType.Exp
    Ln = mybir.ActivationFunctionType.Ln
    Square = mybir.ActivationFunctionType.Square
    Sqrt = mybir.ActivationFunctionType.Sqrt

    nc = bacc.Bacc()

    xn = nc.dram_tensor("xn", [KL, D], bf16, kind="ExternalInput")
    wT = {}
    for br in ("l", "d"):
        for w in ("wq", "wk", "wv", "wo"):
            wT[w, br] = nc.dram_tensor(f"{w}T_{br}", [D, D], bf16,
                                       kind="ExternalInput")
    tri_lo_d = nc.dram_tensor("tri_lo", [P, P], bf16, kind="ExternalInput")
    tri_hi_d = nc.dram_tensor("tri_hi", [P, P], bf16, kind="ExternalInput")
    colmask_d_ = {
        "l": nc.dram_tensor("colmask_l", [P, NKC], f32, kind="ExternalInput"),
        "d": nc.dram_tensor("colmask_d", [P, NKC], f32, kind="ExternalInput"),
    }
    out_dram = {
        "l": nc.dram_tensor("out_l", [QL, D], f32, kind="ExternalOutput"),
        "d": nc.dram_tensor("out_d", [QL, D], f32, kind="ExternalOutput"),
    }

    with tile.TileContext(nc) as tc:
        with (
            tc.tile_pool(name="singles", bufs=1) as singles,
            tc.tile_pool(name="xpool", bufs=3) as xpool,
            tc.tile_pool(name="small", bufs=4) as small,
            tc.tile_pool(name="exppool", bufs=3) as exppool,
            tc.tile_pool(name="outpool", bufs=2) as outpool,
            tc.tile_pool(name="ptr", bufs=2, space="PSUM") as psum_tr,
            tc.tile_pool(name="pproj", bufs=2, space="PSUM") as psum_proj,
            tc.tile_pool(name="pst", bufs=2, space="PSUM") as psum_st,
            tc.tile_pool(name="po", bufs=2, space="PSUM") as psum_o,
        ):
            identity = singles.tile([P, P], bf16)
            make_identity(nc, identity)
            xhatT = singles.tile([P, 4, KL], bf16, name="xhatT")
            eps_t = singles.tile([P, 1], f32, name="eps")
            nc.vector.memset(eps_t, EPS)

            # weights/masks on the gpsimd DGE queue, issued from t=0 in
            # parallel with the x tiles on the sync queue.
            tri_lo = singles.tile([P, P], bf16)
            nc.gpsimd.dma_start(tri_lo, tri_lo_d[:, :])
            tri_hi = singles.tile([P, P], bf16)
            nc.gpsimd.dma_start(tri_hi, tri_hi_d[:, :])
            colmask = {}
            for br in ("l", "d"):
                colmask[br] = singles.tile([P, NKC], f32, name=f"cm_{br}")
                nc.gpsimd.dma_start(colmask[br], colmask_d_[br][:, :])
            w_sb = {}
            for (w, br), dt_ in wT.items():
                w_sb[w, br] = singles.tile([P, 4, D], bf16, name=f"{w}_{br}")
                nc.gpsimd.dma_start(
                    w_sb[w, br],
                    dt_[:, :].rearrange("(ic p) o -> p ic o", p=P),
                )


            # ---- rmsnorm + transpose ----
            for tt in range(NKC):
                xt = xpool.tile([P, D], bf16, tag="xt")
                dma_eng = nc.sync if tt % 2 == 0 else nc.scalar
                dma_eng.dma_start(xt, xn[tt * P:(tt + 1) * P, :])
                sqd = xpool.tile([P, D], bf16, tag="sqd")
                ssum = small.tile([P, 1], f32, tag="ssum")
                nc.scalar.activation(sqd, xt, Square, accum_out=ssum)
                rstd = small.tile([P, 1], f32, tag="rstd")
                nc.scalar.activation(rstd, ssum, Sqrt, bias=eps_t, scale=1.0 / D)
                nc.vector.reciprocal(rstd, rstd)
                xh = xpool.tile([P, D], bf16, tag="xh")
                nc.vector.tensor_scalar_mul(xh, xt, rstd)
                for ic in range(4):
                    tp = psum_tr.tile([P, P], bf16, tag="tp")
                    nc.tensor.transpose(tp, xh[:, ic * P:(ic + 1) * P], identity)
                    nc.vector.tensor_copy(xhatT[:, ic, tt * P:(tt + 1) * P], tp)

            QT, KT, V, OT = {}, {}, {}, {}
            for br in ("l", "d"):
                QT[br] = singles.tile([P, 4, QL], bf16, name=f"QT_{br}")
                KT[br] = singles.tile([P, 4, KL], bf16, name=f"KT_{br}")
                V[br] = singles.tile([P, NKC, H, DH + 1], bf16, name=f"V_{br}")
                OT[br] = singles.tile([P, 4, QL], bf16, name=f"OT_{br}")

            def key_cols_ap(ic, kc, br):
                # lhsT [128, 128] of xhat^T columns for key chunk kc
                if br == "l":
                    return xhatT[:, ic, kc * P:(kc + 1) * P]
                rho, s = kc // 2, kc % 2
                return xhatT[:, ic, :].rearrange(
                    "p (b four) -> p four b", four=DIL)[:, rho, s * P:(s + 1) * P]

            # ---- projections ----
            for br in ("l", "d"):
                for pair in range(4):
                    ps = psum_proj.tile([P, D], f32, tag="pp")
                    for ic in range(4):
                        nc.tensor.matmul(
                            ps, w_sb["wq", br][:, ic, pair * P:(pair + 1) * P],
                            xhatT[:, ic, WIN:WIN + QL],
                            start=(ic == 0), stop=(ic == 3))
                    nc.vector.tensor_copy(QT[br][:, pair, :], ps)
                for pair in range(4):
                    for half in range(2):
                        ps = psum_proj.tile([P, D], f32, tag="pp")
                        for ic in range(4):
                            nc.tensor.matmul(
                                ps, w_sb["wk", br][:, ic, pair * P:(pair + 1) * P],
                                xhatT[:, ic, half * D:(half + 1) * D],
                                start=(ic == 0), stop=(ic == 3))
                        nc.vector.tensor_copy(
                            KT[br][:, pair, half * D:(half + 1) * D], ps)
                for kc in range(NKC):
                    ps = psum_proj.tile([P, D], f32, tag="pp")
                    for ic in range(4):
                        nc.tensor.matmul(
                            ps, key_cols_ap(ic, kc, br), w_sb["wv", br][:, ic, :],
                            start=(ic == 0), stop=(ic == 3))
                    nc.scalar.copy(
                        V[br][:, kc, :, 0:DH],
                        ps.rearrange("p (h dv) -> p h dv", h=H))
                    nc.vector.memset(V[br][:, kc, :, DH:DH + 1], 1.0)
                    nc.vector.tensor_scalar_mul(
                        V[br][:, kc], V[br][:, kc], colmask[br][:, kc:kc + 1])

            # ---- attention ----
            # scores^T per key-chunk -> exp -> {0,1} triangle masks -> O^T =
            # [V | colmask]^T @ expS (PE accumulates straight into the
            # transposed-output layout Wo wants; row 64 is the softmax
            # denominator).  Denominator reciprocal is broadcast across the
            # 64 dv partitions with a DRAM-bounce DMA.
            for br in ("l", "d"):
                for h in range(H):
                    r0, pair = 64 * (h % 2), h // 2
                    if br == "l":
                        ex = exppool.tile([P, NKC, QL], bf16, tag="exp")
                        for kc in range(NKC):
                            qlo = max(0, P * (kc - 4))
                            qhi = min(QL, P * kc + P)
                            n = qhi - qlo
                            st = psum_st.tile([P, QL], f32, tag="st")
                            nc.tensor.matmul(
                                st[:, :n],
                                KT[br][r0:r0 + 64, pair, kc * P:(kc + 1) * P],
                                QT[br][r0:r0 + 64, pair, qlo:qhi])
                            nc.scalar.activation(ex[:, kc, qlo:qhi], st[:, :n], Exp)
                        # edge triangle masks: chunk kc==qtile -> tri_lo at
                        # q-offset 128*kc; chunk kc==qtile+4 -> tri_hi.
                        # Both merged into one strided 3D op over 4 chunks.
                        for kc in range(4):
                            sl = ex[:, kc, P * kc:P * kc + P]
                            nc.vector.tensor_mul(sl, sl, tri_lo)
                        for kc in range(4, NKC):
                            sl = ex[:, kc, P * (kc - 4):P * (kc - 4) + P]
                            nc.vector.tensor_mul(sl, sl, tri_hi)
                        for t in range(NQT):
                            op = psum_o.tile([P, DH + 1], f32, tag="op")
                            for r in range(5):
                                kc = t + r
                                nc.tensor.matmul(
                                    op, ex[:, kc, t * P:(t + 1) * P],
                                    V[br][:, kc, h, :],
                                    start=(r == 0), stop=(r == 4))
                            rcp = small.tile([P, 1], f32, tag="rcp")
                            nc.vector.reciprocal(rcp, op[:, DH:DH + 1])
                            osb = small.tile([P, DH], bf16, tag="osb")
                            nc.vector.tensor_scalar_mul(osb, op[:, 0:DH], rcp)
                            tp = psum_tr.tile([P, P], bf16, tag="tp")
                            nc.tensor.transpose(tp[:DH, :], osb, identity)
                            nc.vector.tensor_copy(
                                OT[br][r0:r0 + 64, pair, t * P:(t + 1) * P],
                                tp[:DH, :])
                    else:
                        ex = exppool.tile([P, NKC, P], bf16, tag="expd")
                        # 4 score chunks share one PSUM tile so exp runs as
                        # one wide ACT op instead of 4 narrow ones.
                        for half in range(2):
                            st = psum_st.tile([P, QL], f32, tag="st")
                            for j in range(4):
                                idx = half * 4 + j
                                rho, s = idx // 2, idx % 2
                                ktv = KT[br][r0:r0 + 64, pair, :].rearrange(
                                    "p (b four) -> p four b", four=DIL
                                )[:, rho, s * P:(s + 1) * P]
                                qtv = QT[br][r0:r0 + 64, pair, :].rearrange(
                                    "p (a four) -> p four a", four=DIL)[:, rho, :]
                                nc.tensor.matmul(st[:, j * P:(j + 1) * P],
                                                 ktv, qtv)
                            nc.scalar.activation(
                                ex[:, half * 4:(half + 1) * 4, :], st, Exp)
                        for rho in range(DIL):
                            for s in range(2):
                                sl = ex[:, rho * 2 + s, :]
                                nc.vector.tensor_mul(
                                    sl, sl, tri_lo if s == 0 else tri_hi)
                        for rho in range(DIL):
                            op = psum_o.tile([P, DH + 1], f32, tag="op")
                            for s in range(2):
                                nc.tensor.matmul(
                                    op, ex[:, rho * 2 + s, :],
                                    V[br][:, rho * 2 + s, h, :],
                                    start=(s == 0), stop=(s == 1))
                            rcp = small.tile([P, 1], f32, tag="rcp")
                            nc.vector.reciprocal(rcp, op[:, DH:DH + 1])
                            osb = small.tile([P, DH], bf16, tag="osb")
                            nc.vector.tensor_scalar_mul(osb, op[:, 0:DH], rcp)
                            tp = psum_tr.tile([P, P], bf16, tag="tp")
                            nc.tensor.transpose(tp[:DH, :], osb, identity)
                            nc.vector.tensor_copy(
                                OT[br][r0:r0 + 64, pair, rho * P:(rho + 1) * P],
                                tp[:DH, :])

                # ---- Wo ----
                for t in range(NQT):
                    ps = psum_proj.tile([P, D], f32, tag="pp")
                    for pair in range(4):
                        nc.tensor.matmul(
                            ps, OT[br][:, pair, t * P:(t + 1) * P],
                            w_sb["wo", br][:, pair, :],
                            start=(pair == 0), stop=(pair == 3))
                    ob = outpool.tile([P, D], f32, tag="ob")
                    nc.scalar.copy(ob, ps)
                    if br == "l":
                        nc.sync.dma_start(out_dram[br][t * P:(t + 1) * P, :], ob)
                    else:
                        dst = out_dram[br][:, :].rearrange(
                            "(a four) o -> four a o", four=DIL)[t]
                        nc.sync.dma_start(dst, ob)

    nc.finalize()
    return nc


def _prep_host(x, key_padding_mask, weights):
    """Build the per-core input maps (weights shared across cores)."""
    x = np.asarray(x, dtype=np.float32).reshape(L, D)
    kpm = np.asarray(key_padding_mask).reshape(L).astype(bool)

    shared = {}
    for name, arr in weights.items():
        shared[name] = np.ascontiguousarray(arr.T).astype(BF16)

    idx = np.arange(P)
    tri_lo = (idx[:, None] >= idx[None, :]).astype(BF16)
    tri_hi = (idx[:, None] <= idx[None, :]).astype(BF16)
    shared["tri_lo"], shared["tri_hi"] = tri_lo, tri_hi

    valid_full = np.zeros(L + 2 * WIN, dtype=np.float32)
    valid_full[WIN:WIN + L] = (~kpm).astype(np.float32)

    in_maps = []
    for c in range(NCORES):
        lo = c * QL - WIN
        xnc = np.zeros((KL, D), dtype=np.float32)
        a, b = max(lo, 0), min(lo + KL, L)
        xnc[a - lo:b - lo] = x[a:b]
        v = valid_full[lo + WIN:lo + WIN + KL]  # validity of keys lo..lo+KL
        cm_l = v.reshape(NKC, P).T.astype(np.float32)
        # dilated chunk idx = rho*2+s holds keys lk = 4*(128*s + p) + rho
        cm_d = np.empty((P, NKC), dtype=np.float32)
        for rho in range(DIL):
            for s in range(2):
                lk = DIL * (P * s + idx) + rho
                cm_d[:, rho * 2 + s] = v[lk]
        m = dict(shared)
        m["xn"] = xnc.astype(BF16)
        m["colmask_l"] = np.ascontiguousarray(cm_l)
        m["colmask_d"] = np.ascontiguousarray(cm_d)
        in_maps.append(m)
    return in_maps


def kernel(x, key_padding_mask, wq_l, wk_l, wv_l, wo_l,
           wq_d, wk_d, wv_d, wo_d, g_q, g_kv, **run_kwargs):
    from concourse.bass_utils import run_bass_kernel_spmd

    g_q = np.asarray(g_q, dtype=np.float32)
    g_kv = np.asarray(g_kv, dtype=np.float32)
    scale = 1.0 / np.sqrt(DH)
    weights = {
        "wqT_l": np.asarray(wq_l, np.float32) * (g_q * scale)[None, :],
        "wkT_l": np.asarray(wk_l, np.float32) * g_kv[None, :],
        "wvT_l": np.asarray(wv_l, np.float32) * g_kv[None, :],
        "woT_l": np.asarray(wo_l, np.float32),
        "wqT_d": np.asarray(wq_d, np.float32) * (g_q * scale)[None, :],
        "wkT_d": np.asarray(wk_d, np.float32) * g_kv[None, :],
        "wvT_d": np.asarray(wv_d, np.float32) * g_kv[None, :],
        "woT_d": np.asarray(wo_d, np.float32),
    }
    in_maps = _prep_host(x, key_padding_mask, weights)

    if "nc" not in _STATE:
        _STATE["nc"] = _build_nc()
    res = run_bass_kernel_spmd(_STATE["nc"], in_maps,
                               core_ids=list(range(NCORES)), **run_kwargs)
    _STATE["last_result"] = res

    out_l = np.concatenate([res.results[c]["out_l"] for c in range(NCORES)],
                           axis=0).reshape(1, L, D)
    out_d = np.concatenate([res.results[c]["out_d"] for c in range(NCORES)],
                           axis=0).reshape(1, L, D)
    return (out_l, out_d)

